# revision 26
# baseline (speedup 1.0000x reference)
"""Trainium2 Bass kernel for nn_AttentionHead (B=4, S=4096, H=1024, D=64).

Reference computation (note the unusual K-first ordering):
    K = x @ Wk.T; Q = x @ Wq.T; V = x @ Wv.T            [B,S,D]
    scores[b,i,j] = (K[b,i] . Q[b,j]) / sqrt(D)         [B,S,S]
    scores[:, :, j] = -1e12 where mask[:, j] == 0
    out = softmax(scores, axis=2) @ V                   [B,S,D]

Sharding: 8 cores = 4 batches x 2 key-row chunks of 2048. The softmax axis
is the QUERY axis j, and mask==0 kills column j outright (weight exactly 0
for every output row). ~50% of positions are masked, so the host gathers
only the unmasked query columns (padded to U=2176 with -30000-bias slots
that exp to exactly 0) — halving Q/V projection, scores, exp and AV work.
The host also pre-casts x to bf16 and pre-transposes it, so x streams in
ready for the PE's contraction layout (no on-device casts or transposes).

Per-core pipeline (bf16 matmuls, fp32 accumulation):
  - x^T key-slice [1024, 2048] and gathered query-slice [1024, 2176] DMA
    into SBUF across the three DMA queues (sync/scalar/gpsimd), ordered so
    the first-needed chunks land first.
  - K^T is projected with a duplicated stationary [Wk.T | Wk.T] so rows
    0:64 AND 64:128 of kt both hold K^T: the scores matmul has contraction
    K=D=64, so two independent 64-row matmuls run CONCURRENTLY on the two
    halves of the 128x128 PE array (tile_position auto-derived from base
    partitions) — 2x scores throughput. Q^T is likewise duplicated via a
    second DVE copy. One [Wq.T | Wv.T] stationary gives Q^T and V^T
    together; V^T -> V via PE transposes (with a ones column appended for
    the softmax denominator).
  - Two passes over 1024-wide key halves. Per query tile t: packed
    scores^T = Q^T_t.T @ K^T on PE; exp(0.125*s + padbias[j]) on ACT; PE
    accumulates V'_t.T @ P^T_t into out'^T [65, 1024] (rows 0:64
    numerator^T, row 64 denominator). The AV matmuls are emitted TWO
    slots behind the scores matmuls so they never wait on exp (one slot
    behind still ping-pongs PE<->ACT at ~300ns/slot).
  - Per-pass finale: PE-transpose out'^T, then numerator * reciprocal
    (denominator) on DVE; one DMA store per pass.
"""

import numpy as np

B, S, H, D = 4, 4096, 1024, 64
N_CORES = 8
SC = S // 2  # key rows per core
HC = H // 128  # contraction chunks
U = 2176  # padded unmasked query capacity (17 tiles of 128)
UT = U // 128  # query tiles
NEG = -30000.0
N_WARM = 12

_CACHE = {}


def _build():
    import concourse.bass as bass
    import concourse.tile as tile
    from concourse import bacc, mybir

    dt = mybir.dt
    AF = mybir.ActivationFunctionType

    nc = bacc.Bacc(
        "TRN2", target_bir_lowering=False, debug=False, num_devices=N_CORES
    )
    # x chunks arrive pre-transposed and pre-tiled [128, HC, W] so every DMA
    # is per-partition contiguous (128 descriptors, 8KB lines) — the
    # rearranged-AP version generated 1024 descriptors per chunk and was
    # descriptor-bound at ~160 GB/s with a multi-us issue cost per chunk.
    # chunk 0 of xk/xq carries the (tiny) weight matrices as 128 extra
    # columns so they ride a big-line DMA — standalone 2KB-per-partition
    # weight DMAs measured ~60 GB/s (per-descriptor-bound) and delayed the
    # first projection by ~10us.
    xk = [
        nc.dram_tensor(
            f"xk{k}", [128, HC, 640 if k == 0 else 512], dt.bfloat16,
            kind="ExternalInput",
        ).ap()
        for k in range(4)
    ]
    xq = [
        nc.dram_tensor(
            f"xq{j}", [128, HC, 640 if j == 0 else (512 if j < 4 else U - 2048)],
            dt.bfloat16, kind="ExternalInput",
        ).ap()
        for j in range(5)
    ]
    mb = nc.dram_tensor("mb", [128, UT], dt.float32, kind="ExternalInput").ap()
    identb = nc.dram_tensor("identb", [128, 128], dt.bfloat16, kind="ExternalInput").ap()
    identf = nc.dram_tensor("identf", [128, 128], dt.float32, kind="ExternalInput").ap()
    out = nc.dram_tensor("out", [128, 16, D], dt.float32, kind="ExternalOutput").ap()

    with (
        tile.TileContext(nc) as tc,
        tc.tile_pool(name="persist", bufs=1) as persist,
        tc.tile_pool(name="ptile", bufs=6) as ptile,
        tc.tile_pool(name="accs", bufs=2) as accs,
        tc.tile_pool(name="fin", bufs=2) as fin,
    ):
        xk_sb = [
            persist.tile(
                [128, HC, 640 if k == 0 else 512], dt.bfloat16, name=f"xk_sb{k}"
            )
            for k in range(4)
        ]
        xq_sb = [
            persist.tile(
                [128, HC, 640 if j == 0 else (512 if j < 4 else U - 2048)],
                dt.bfloat16, name=f"xq_sb{j}",
            )
            for j in range(5)
        ]
        qt = persist.tile([128, U], dt.bfloat16)  # Q^T duplicated on both halves
        kt = persist.tile([128, SC], dt.bfloat16)  # K^T duplicated on both halves
        vtsb = persist.tile([128, U], dt.bfloat16)  # rows 64:128 = V^T
        vp = persist.tile([128, UT, D + 1], dt.bfloat16)
        mb_sb = persist.tile([128, UT], dt.float32)
        id_bf = persist.tile([128, 128], dt.bfloat16)
        id_f32 = persist.tile([128, 128], dt.float32)
        junk = persist.tile([128, 512], dt.bfloat16)

        nc.vector.memset(vp[:, :, D], 1.0)
        nc.vector.memset(junk[:], 0.0)

        # --- DMA schedule: bulk data on the two HWDGE queues (sync/scalar),
        # tiny tensors on the slow gpsimd software-DGE queue ---
        nc.sync.dma_start(xk_sb[0][:], xk[0][:])
        nc.sync.dma_start(xk_sb[1][:], xk[1][:])
        nc.sync.dma_start(xk_sb[2][:], xk[2][:])
        nc.sync.dma_start(xk_sb[3][:], xk[3][:])
        nc.sync.dma_start(xq_sb[4][:], xq[4][:])

        nc.scalar.dma_start(xq_sb[0][:], xq[0][:])
        nc.scalar.dma_start(xq_sb[1][:], xq[1][:])
        nc.scalar.dma_start(xq_sb[2][:], xq[2][:])
        nc.scalar.dma_start(xq_sb[3][:], xq[3][:])

        nc.gpsimd.dma_start(mb_sb[:], mb[:])
        nc.gpsimd.dma_start(id_bf[:], identb[:])
        nc.gpsimd.dma_start(id_f32[:], identf[:])

        with (
            tc.tile_pool(name="psco", bufs=2, space="PSUM") as psco,
            tc.tile_pool(name="ppx", bufs=2, space="PSUM") as ppx,
            tc.tile_pool(name="pacc", bufs=1, space="PSUM") as pacc,
        ):

            def qv_mms(j, ps, h0, h1):  # hc sub-range of block j's projection
                c0 = 512 * j
                w = min(512, U - c0)
                for hc in range(h0, h1):
                    nc.tensor.matmul(
                        ps[:, 0:w],
                        xq_sb[0][:, hc, 512:640],
                        xq_sb[j][:, hc, 0:w],
                        start=(hc == 0),
                        stop=(hc == HC - 1),
                    )

            def qv_copies(j, ps, nsp=1):
                c0 = 512 * j
                w = min(512, U - c0)
                for s in range(nsp):
                    a, b = s * w // nsp, (s + 1) * w // nsp
                    nc.vector.tensor_copy(qt[0:64, c0 + a : c0 + b], ps[0:64, a:b])
                    nc.vector.tensor_copy(qt[64:128, c0 + a : c0 + b], ps[0:64, a:b])
                    nc.vector.tensor_copy(
                        vtsb[64:128, c0 + a : c0 + b], ps[64:128, a:b]
                    )

            def proj_qv(j, nsp=1):
                ps = ppx.tile([128, 512], dt.float32, tag="px")
                qv_mms(j, ps, 0, HC)
                qv_copies(j, ps, nsp)

            def k_mms(sb, ps, h0, h1):
                for hc in range(h0, h1):
                    nc.tensor.matmul(
                        ps[:],
                        xk_sb[0][:, hc, 512:640],
                        xk_sb[sb][:, hc, 0:512],
                        start=(hc == 0),
                        stop=(hc == HC - 1),
                    )

            def k_copy(sb, ps):
                nc.vector.tensor_copy(kt[:, 512 * sb : 512 * (sb + 1)], ps[:])

            def proj_k(sb):
                ps = ppx.tile([128, 512], dt.float32, tag="px")
                k_mms(sb, ps, 0, HC)
                k_copy(sb, ps)

            def vt_block(t0, t1):  # V^T -> V via PE transpose
                for t in range(t0, t1):
                    pvt = ppx.tile([128, D], dt.bfloat16, tag="px")
                    nc.tensor.transpose(
                        pvt[:],
                        vtsb[64:128, 128 * t : 128 * (t + 1)],
                        id_bf[64:128, 64:128],
                    )
                    nc.vector.tensor_copy(vp[:, t, 0:D], pvt[:])

            # --- t-loop slot machinery: AV deferred TWO slots behind so the
            # AV matmuls never wait on exp (which would stall the next
            # scores pair and ping-pong PE<->ACT at ~300ns/slot) ---
            pending = []

            def flush_av(acc):
                if not pending:
                    return
                pt, t = pending.pop(0)
                for nb in range(2):
                    nc.tensor.matmul(
                        acc[:, 512 * nb : 512 * (nb + 1)],
                        vp[:, t, :],
                        pt[:, 512 * nb : 512 * (nb + 1)],
                        start=(t == 0),
                        stop=(t == UT - 1),
                    )

            def t_slot(t, acc, ih):
                # packed scores: two concurrent 64-contraction matmuls on
                # the two halves of the PE array (rows 0:64 / 64:128)
                ps = psco.tile([128, 1024], dt.float32, tag="ps")
                k0 = 1024 * ih
                if len(pending) >= 2:
                    flush_av(acc)
                nc.tensor.matmul(
                    ps[:, 0:512],
                    qt[0:64, 128 * t : 128 * (t + 1)],
                    kt[0:64, k0 : k0 + 512],
                    start=True,
                    stop=True,
                )
                nc.tensor.matmul(
                    ps[:, 512:1024],
                    qt[64:128, 128 * t : 128 * (t + 1)],
                    kt[64:128, k0 + 512 : k0 + 1024],
                    start=True,
                    stop=True,
                )
                pt = ptile.tile([128, 1024], dt.bfloat16)
                nc.scalar.activation(
                    pt[:], ps[:], AF.Exp, bias=mb_sb[:, t : t + 1], scale=0.125
                )
                pending.append((pt, t))

            def finale(acc_sb, ih):
                for k in range(8):
                    po = ppx.tile([128, D + 1], dt.float32, tag="px")
                    nc.tensor.transpose(
                        po[:],
                        acc_sb[:, 128 * k : 128 * (k + 1)],
                        id_f32[0 : D + 1, 0 : D + 1],
                    )
                    rc = fin.tile([128, 1], dt.float32, tag="rc")
                    nc.vector.reciprocal(rc[:], po[:, D : D + 1])
                    nc.vector.tensor_scalar_mul(
                        oall[:, 8 * ih + k, :], po[:, 0:D], rc[:]
                    )

            oall = fin.tile([128, 16, D], dt.float32, tag="oall")

            # PE warmup on junk data while the first DMAs stream in
            pw = ppx.tile([128, 512], dt.float32, tag="px")
            for _ in range(N_WARM):
                nc.tensor.matmul(
                    pw[:], junk[:, 0:128], junk[:], start=True, stop=True
                )
            dummy = fin.tile([128, 1], dt.float32, tag="dummy")
            nc.scalar.activation(dummy[:], junk[:, 0:1], AF.Exp)

            # ---- projections: front-load what the DMA can deliver before
            # the t-loop; later blocks interleave at the slots where their
            # DMA chunks actually land. NOTE: keeping the PE dense with
            # filler matmuls (to hold the HAM clock-gate at 2.4 GHz) was
            # tried and REGRESSED: with all 8 cores dense, the package
            # drops to a lower power state and every engine slows ~17%,
            # which hurts more than the idle-time HAM throttle does. ----
            accA = pacc.tile([D + 1, 1024], dt.float32, tag="acc")

            proj_k(0)
            proj_k(1)
            proj_qv(0, nsp=2)
            vt_block(0, 4)
            proj_k(2)

            # ---- pass A (keys 0:1024); late proj blocks interleave at the
            # slots where their DMA chunks have actually landed (hc-granular
            # spreading was tried and measured neutral-to-worse: the stalls
            # are partly DMA-gated and spreading taxes every slot) ----
            proj_qv(1)
            vt_block(4, 8)
            proj_qv(2)
            vt_block(8, 12)
            for t in range(0, 9):
                t_slot(t, accA, 0)
            proj_qv(3)
            t_slot(9, accA, 0)
            vt_block(12, 16)
            t_slot(10, accA, 0)
            proj_k(3)
            t_slot(11, accA, 0)
            t_slot(12, accA, 0)
            proj_qv(4)
            vt_block(16, 17)
            for t in range(13, UT):
                t_slot(t, accA, 0)
            flush_av(accA)
            flush_av(accA)
            acc_sbA = accs.tile([D + 1, 1024], dt.float32, tag="accs")
            nc.vector.tensor_copy(acc_sbA[:, 0:512], accA[:, 0:512])
            nc.vector.tensor_copy(acc_sbA[:, 512:1024], accA[:, 512:1024])

            # ---- finale A + pass B (keys 1024:2048) ----
            finale(acc_sbA, 0)
            nc.sync.dma_start(out[:, 0:8, :], oall[:, 0:8, :])
            accB = pacc.tile([D + 1, 1024], dt.float32, tag="acc")
            for t in range(UT):
                t_slot(t, accB, 1)
            flush_av(accB)
            flush_av(accB)
            acc_sbB = accs.tile([D + 1, 1024], dt.float32, tag="accs")
            nc.vector.tensor_copy(acc_sbB[:, 0:512], accB[:, 0:512])
            nc.vector.tensor_copy(acc_sbB[:, 512:1024], accB[:, 512:1024])
            finale(acc_sbB, 1)
            nc.sync.dma_start(out[:, 8:16, :], oall[:, 8:16, :])

    nc.compile()
    return nc


def _tile_pcs(xt):
    """[H, W] -> [128, HC, W] with pcs[p, c, :] = xt[c*128 + p, :], contiguous."""
    W = xt.shape[1]
    return np.ascontiguousarray(xt.reshape(HC, 128, W).transpose(1, 0, 2))


def _in_maps(x, mask, Wk, Wq, Wv):
    import ml_dtypes

    bf16 = ml_dtypes.bfloat16
    wqv = _tile_pcs(np.concatenate([Wq.T, Wv.T], axis=1).astype(bf16))
    wk2 = _tile_pcs(np.concatenate([Wk.T, Wk.T], axis=1).astype(bf16))
    identb = np.eye(128, dtype=np.float32).astype(bf16)
    identf = np.eye(128, dtype=np.float32)
    maps = []
    for b in range(B):
        idx = np.nonzero(mask[b])[0]
        u = len(idx)
        assert u <= U, f"unmasked count {u} exceeds padded capacity {U}"
        idx_pad = np.concatenate([idx, np.full(U - u, idx[0], dtype=np.int64)])
        xqt_ = _tile_pcs(x[b][idx_pad].astype(bf16).T)  # [128, HC, U]
        qchunks = {
            f"xq{j}": np.ascontiguousarray(xqt_[:, :, 512 * j : min(512 * (j + 1), U)])
            for j in range(1, 5)
        }
        # chunk 0 carries [Wq.T | Wv.T] as 128 extra columns
        qchunks["xq0"] = np.ascontiguousarray(
            np.concatenate([xqt_[:, :, 0:512], wqv], axis=2)
        )
        mbv = np.zeros(U, dtype=np.float32)
        mbv[u:] = NEG
        mbt = np.ascontiguousarray(mbv.reshape(UT, 128).T)  # [128, UT]
        for half in range(2):
            xkt_ = _tile_pcs(
                x[b, half * SC : (half + 1) * SC].astype(bf16).T
            )  # [128, HC, SC]
            m = {
                f"xk{k}": np.ascontiguousarray(xkt_[:, :, 512 * k : 512 * (k + 1)])
                for k in range(1, 4)
            }
            # chunk 0 carries [Wk.T | Wk.T] as 128 extra columns
            m["xk0"] = np.ascontiguousarray(
                np.concatenate([xkt_[:, :, 0:512], wk2], axis=2)
            )
            m.update(qchunks)
            m.update({"mb": mbt, "identb": identb, "identf": identf})
            maps.append(m)
    return maps


def kernel(x, mask, Wk, Wq, Wv):
    from concourse.bass_utils import run_bass_kernel_spmd

    if "nc" not in _CACHE:
        _CACHE["nc"] = _build()
    nc = _CACHE["nc"]
    maps = _in_maps(x, mask, Wk, Wq, Wv)
    br = run_bass_kernel_spmd(nc, maps, list(range(N_CORES)))
    out = np.empty((B, S, D), dtype=np.float32)
    for c in range(N_CORES):
        b, half = c // 2, c % 2
        # device layout [128, 16, D]: row 128*k + p lives at [p, k, :]
        o = br.results[c]["out"].transpose(1, 0, 2).reshape(SC, D)
        out[b, half * SC : (half + 1) * SC, :] = o
    return out


# revision 27
# speedup vs baseline: 1.0445x; 1.0445x over previous
"""Trainium2 Bass kernel for nn_AttentionHead (B=4, S=4096, H=1024, D=64).

Reference computation (note the unusual K-first ordering):
    K = x @ Wk.T; Q = x @ Wq.T; V = x @ Wv.T            [B,S,D]
    scores[b,i,j] = (K[b,i] . Q[b,j]) / sqrt(D)         [B,S,S]
    scores[:, :, j] = -1e12 where mask[:, j] == 0
    out = softmax(scores, axis=2) @ V                   [B,S,D]

Sharding: 8 cores = 4 batches x 2 key-row chunks of 2048. The softmax axis
is the QUERY axis j, and mask==0 kills column j outright (weight exactly 0
for every output row). ~50% of positions are masked, so the host gathers
only the unmasked query columns (padded to U=2176 with -30000-bias slots
that exp to exactly 0) — halving Q/V projection, scores, exp and AV work.
The host also pre-casts x to bf16 and pre-transposes it, so x streams in
ready for the PE's contraction layout (no on-device casts or transposes).

Per-core pipeline (bf16 matmuls, fp32 accumulation):
  - x^T key-slice [1024, 2048] and gathered query-slice [1024, 2176] DMA
    into SBUF across the three DMA queues (sync/scalar/gpsimd), ordered so
    the first-needed chunks land first.
  - K^T is projected with a duplicated stationary [Wk.T | Wk.T] so rows
    0:64 AND 64:128 of kt both hold K^T: the scores matmul has contraction
    K=D=64, so two independent 64-row matmuls run CONCURRENTLY on the two
    halves of the 128x128 PE array (tile_position auto-derived from base
    partitions) — 2x scores throughput. Q^T is likewise duplicated via a
    second DVE copy. One [Wq.T | Wv.T] stationary gives Q^T and V^T
    together; V^T -> V via PE transposes (with a ones column appended for
    the softmax denominator).
  - Two passes over 1024-wide key halves. Per query tile t: packed
    scores^T = Q^T_t.T @ K^T on PE; exp(0.125*s + padbias[j]) on ACT; PE
    accumulates V'_t.T @ P^T_t into out'^T [65, 1024] (rows 0:64
    numerator^T, row 64 denominator). The AV matmuls are emitted TWO
    slots behind the scores matmuls so they never wait on exp (one slot
    behind still ping-pongs PE<->ACT at ~300ns/slot).
  - Per-pass finale: PE-transpose out'^T, then numerator * reciprocal
    (denominator) on DVE; one DMA store per pass.
"""

import numpy as np

B, S, H, D = 4, 4096, 1024, 64
N_CORES = 8
SC = S // 2  # key rows per core
HC = H // 128  # contraction chunks
U = 2176  # padded unmasked query capacity (17 tiles of 128)
UT = U // 128  # query tiles
NEG = -30000.0
N_WARM = 12

_CACHE = {}


def _build():
    import concourse.bass as bass
    import concourse.tile as tile
    from concourse import bacc, mybir

    dt = mybir.dt
    AF = mybir.ActivationFunctionType

    nc = bacc.Bacc(
        "TRN2", target_bir_lowering=False, debug=False, num_devices=N_CORES
    )
    # x chunks arrive pre-transposed and pre-tiled [128, HC, W] so every DMA
    # is per-partition contiguous (128 descriptors, 8KB lines) — the
    # rearranged-AP version generated 1024 descriptors per chunk and was
    # descriptor-bound at ~160 GB/s with a multi-us issue cost per chunk.
    # chunk 0 of xk/xq carries the (tiny) weight matrices as 128 extra
    # columns so they ride a big-line DMA — standalone 2KB-per-partition
    # weight DMAs measured ~60 GB/s (per-descriptor-bound) and delayed the
    # first projection by ~10us.
    xk = [
        nc.dram_tensor(
            f"xk{k}", [128, HC, 640 if k == 0 else 512], dt.bfloat16,
            kind="ExternalInput",
        ).ap()
        for k in range(4)
    ]
    xq = [
        nc.dram_tensor(
            f"xq{j}", [128, HC, 640 if j == 0 else (512 if j < 4 else U - 2048)],
            dt.bfloat16, kind="ExternalInput",
        ).ap()
        for j in range(5)
    ]
    mb = nc.dram_tensor("mb", [128, UT], dt.float32, kind="ExternalInput").ap()
    identb = nc.dram_tensor("identb", [128, 128], dt.bfloat16, kind="ExternalInput").ap()
    identf = nc.dram_tensor("identf", [128, 128], dt.float32, kind="ExternalInput").ap()
    out = nc.dram_tensor("out", [128, 16, D], dt.float32, kind="ExternalOutput").ap()

    with (
        tile.TileContext(nc) as tc,
        tc.tile_pool(name="persist", bufs=1) as persist,
        tc.tile_pool(name="ptile", bufs=6) as ptile,
        tc.tile_pool(name="accs", bufs=2) as accs,
        tc.tile_pool(name="fin", bufs=2) as fin,
    ):
        xk_sb = [
            persist.tile(
                [128, HC, 640 if k == 0 else 512], dt.bfloat16, name=f"xk_sb{k}"
            )
            for k in range(4)
        ]
        xq_sb = [
            persist.tile(
                [128, HC, 640 if j == 0 else (512 if j < 4 else U - 2048)],
                dt.bfloat16, name=f"xq_sb{j}",
            )
            for j in range(5)
        ]
        qt = persist.tile([128, U], dt.bfloat16)  # Q^T duplicated on both halves
        kt = persist.tile([128, SC], dt.bfloat16)  # K^T duplicated on both halves
        vtsb = persist.tile([128, U], dt.bfloat16)  # rows 64:128 = V^T
        vp = persist.tile([128, UT, D + 1], dt.bfloat16)
        mb_sb = persist.tile([128, UT], dt.float32)
        id_bf = persist.tile([128, 128], dt.bfloat16)
        id_f32 = persist.tile([128, 128], dt.float32)
        junk = persist.tile([128, 512], dt.bfloat16)

        nc.vector.memset(vp[:, :, D], 1.0)
        nc.vector.memset(junk[:], 0.0)

        # --- DMA schedule: bulk data on the two HWDGE queues (sync/scalar),
        # tiny tensors on the slow gpsimd software-DGE queue ---
        nc.sync.dma_start(xk_sb[0][:], xk[0][:])
        nc.sync.dma_start(xk_sb[1][:], xk[1][:])
        nc.sync.dma_start(xq_sb[1][:], xq[1][:])
        nc.sync.dma_start(xk_sb[2][:], xk[2][:])
        nc.sync.dma_start(xk_sb[3][:], xk[3][:])
        nc.sync.dma_start(xq_sb[4][:], xq[4][:])

        nc.scalar.dma_start(xq_sb[0][:], xq[0][:])
        nc.scalar.dma_start(xq_sb[2][:], xq[2][:])
        nc.scalar.dma_start(xq_sb[3][:], xq[3][:])

        nc.gpsimd.dma_start(mb_sb[:], mb[:])
        nc.gpsimd.dma_start(id_bf[:], identb[:])
        nc.gpsimd.dma_start(id_f32[:], identf[:])

        with (
            tc.tile_pool(name="psco", bufs=2, space="PSUM") as psco,
            tc.tile_pool(name="ppx", bufs=2, space="PSUM") as ppx,
            tc.tile_pool(name="pacc", bufs=1, space="PSUM") as pacc,
        ):

            def qv_mms(j, ps, h0, h1):  # hc sub-range of block j's projection
                c0 = 512 * j
                w = min(512, U - c0)
                for hc in range(h0, h1):
                    nc.tensor.matmul(
                        ps[:, 0:w],
                        xq_sb[0][:, hc, 512:640],
                        xq_sb[j][:, hc, 0:w],
                        start=(hc == 0),
                        stop=(hc == HC - 1),
                    )

            def qv_copies(j, ps, nsp=1):
                c0 = 512 * j
                w = min(512, U - c0)
                for s in range(nsp):
                    a, b = s * w // nsp, (s + 1) * w // nsp
                    nc.vector.tensor_copy(qt[0:64, c0 + a : c0 + b], ps[0:64, a:b])
                    nc.vector.tensor_copy(qt[64:128, c0 + a : c0 + b], ps[0:64, a:b])
                    nc.vector.tensor_copy(
                        vtsb[64:128, c0 + a : c0 + b], ps[64:128, a:b]
                    )

            def proj_qv(j, nsp=1):
                ps = ppx.tile([128, 512], dt.float32, tag="px")
                qv_mms(j, ps, 0, HC)
                qv_copies(j, ps, nsp)

            def k_mms(sb, ps, h0, h1):
                for hc in range(h0, h1):
                    nc.tensor.matmul(
                        ps[:],
                        xk_sb[0][:, hc, 512:640],
                        xk_sb[sb][:, hc, 0:512],
                        start=(hc == 0),
                        stop=(hc == HC - 1),
                    )

            def k_copy(sb, ps):
                nc.vector.tensor_copy(kt[:, 512 * sb : 512 * (sb + 1)], ps[:])

            def proj_k(sb):
                ps = ppx.tile([128, 512], dt.float32, tag="px")
                k_mms(sb, ps, 0, HC)
                k_copy(sb, ps)

            def vt_block(t0, t1):  # V^T -> V via PE transpose
                for t in range(t0, t1):
                    pvt = ppx.tile([128, D], dt.bfloat16, tag="px")
                    nc.tensor.transpose(
                        pvt[:],
                        vtsb[64:128, 128 * t : 128 * (t + 1)],
                        id_bf[64:128, 64:128],
                    )
                    nc.vector.tensor_copy(vp[:, t, 0:D], pvt[:])

            # --- t-loop slot machinery: AV deferred TWO slots behind so the
            # AV matmuls never wait on exp (which would stall the next
            # scores pair and ping-pong PE<->ACT at ~300ns/slot) ---
            pending = []

            def flush_av(acc):
                if not pending:
                    return
                pt, t = pending.pop(0)
                for nb in range(2):
                    nc.tensor.matmul(
                        acc[:, 512 * nb : 512 * (nb + 1)],
                        vp[:, t, :],
                        pt[:, 512 * nb : 512 * (nb + 1)],
                        start=(t == 0),
                        stop=(t == UT - 1),
                    )

            def t_slot(t, acc, ih):
                # packed scores: two concurrent 64-contraction matmuls on
                # the two halves of the PE array (rows 0:64 / 64:128)
                ps = psco.tile([128, 1024], dt.float32, tag="ps")
                k0 = 1024 * ih
                if len(pending) >= 2:
                    flush_av(acc)
                nc.tensor.matmul(
                    ps[:, 0:512],
                    qt[0:64, 128 * t : 128 * (t + 1)],
                    kt[0:64, k0 : k0 + 512],
                    start=True,
                    stop=True,
                )
                nc.tensor.matmul(
                    ps[:, 512:1024],
                    qt[64:128, 128 * t : 128 * (t + 1)],
                    kt[64:128, k0 + 512 : k0 + 1024],
                    start=True,
                    stop=True,
                )
                pt = ptile.tile([128, 1024], dt.bfloat16)
                nc.scalar.activation(
                    pt[:], ps[:], AF.Exp, bias=mb_sb[:, t : t + 1], scale=0.125
                )
                pending.append((pt, t))

            def finale(acc_sb, ih):
                for k in range(8):
                    po = ppx.tile([128, D + 1], dt.float32, tag="px")
                    nc.tensor.transpose(
                        po[:],
                        acc_sb[:, 128 * k : 128 * (k + 1)],
                        id_f32[0 : D + 1, 0 : D + 1],
                    )
                    rc = fin.tile([128, 1], dt.float32, tag="rc")
                    nc.vector.reciprocal(rc[:], po[:, D : D + 1])
                    nc.vector.tensor_scalar_mul(
                        oall[:, 8 * ih + k, :], po[:, 0:D], rc[:]
                    )

            oall = fin.tile([128, 16, D], dt.float32, tag="oall")

            # PE warmup on junk data while the first DMAs stream in
            pw = ppx.tile([128, 512], dt.float32, tag="px")
            for _ in range(N_WARM):
                nc.tensor.matmul(
                    pw[:], junk[:, 0:128], junk[:], start=True, stop=True
                )
            dummy = fin.tile([128, 1], dt.float32, tag="dummy")
            nc.scalar.activation(dummy[:], junk[:, 0:1], AF.Exp)

            # ---- projections: front-load what the DMA can deliver before
            # the t-loop; later blocks interleave at the slots where their
            # DMA chunks actually land. NOTE: keeping the PE dense with
            # filler matmuls (to hold the HAM clock-gate at 2.4 GHz) was
            # tried and REGRESSED: with all 8 cores dense, the package
            # drops to a lower power state and every engine slows ~17%,
            # which hurts more than the idle-time HAM throttle does. ----
            accA = pacc.tile([D + 1, 1024], dt.float32, tag="acc")

            proj_k(0)
            proj_k(1)
            proj_qv(0, nsp=2)
            vt_block(0, 4)

            # ---- pass A (keys 0:1024); late proj blocks interleave at the
            # slots where their DMA chunks have actually landed (hc-granular
            # spreading was tried and measured neutral-to-worse: the stalls
            # are partly DMA-gated and spreading taxes every slot) ----
            t_slot(0, accA, 0)
            proj_qv(1)
            vt_block(4, 8)
            for t in range(1, 4):
                t_slot(t, accA, 0)
            proj_qv(2)
            vt_block(8, 12)
            for t in range(4, 6):
                t_slot(t, accA, 0)
            proj_k(2)
            for t in range(6, 9):
                t_slot(t, accA, 0)
            proj_qv(3)
            t_slot(9, accA, 0)
            vt_block(12, 16)
            for t in range(10, 13):
                t_slot(t, accA, 0)
            proj_k(3)
            t_slot(13, accA, 0)
            proj_qv(4)
            vt_block(16, 17)
            for t in range(14, UT):
                t_slot(t, accA, 0)
            flush_av(accA)
            flush_av(accA)
            acc_sbA = accs.tile([D + 1, 1024], dt.float32, tag="accs")
            nc.vector.tensor_copy(acc_sbA[:, 0:512], accA[:, 0:512])
            nc.vector.tensor_copy(acc_sbA[:, 512:1024], accA[:, 512:1024])

            # ---- finale A + pass B (keys 1024:2048) ----
            finale(acc_sbA, 0)
            nc.sync.dma_start(out[:, 0:8, :], oall[:, 0:8, :])
            accB = pacc.tile([D + 1, 1024], dt.float32, tag="acc")
            for t in range(UT):
                t_slot(t, accB, 1)
            flush_av(accB)
            flush_av(accB)
            acc_sbB = accs.tile([D + 1, 1024], dt.float32, tag="accs")
            nc.vector.tensor_copy(acc_sbB[:, 0:512], accB[:, 0:512])
            nc.vector.tensor_copy(acc_sbB[:, 512:1024], accB[:, 512:1024])
            finale(acc_sbB, 1)
            nc.sync.dma_start(out[:, 8:16, :], oall[:, 8:16, :])

    nc.compile()
    return nc


def _tile_pcs(xt):
    """[H, W] -> [128, HC, W] with pcs[p, c, :] = xt[c*128 + p, :], contiguous."""
    W = xt.shape[1]
    return np.ascontiguousarray(xt.reshape(HC, 128, W).transpose(1, 0, 2))


def _in_maps(x, mask, Wk, Wq, Wv):
    import ml_dtypes

    bf16 = ml_dtypes.bfloat16
    wqv = _tile_pcs(np.concatenate([Wq.T, Wv.T], axis=1).astype(bf16))
    wk2 = _tile_pcs(np.concatenate([Wk.T, Wk.T], axis=1).astype(bf16))
    identb = np.eye(128, dtype=np.float32).astype(bf16)
    identf = np.eye(128, dtype=np.float32)
    maps = []
    for b in range(B):
        idx = np.nonzero(mask[b])[0]
        u = len(idx)
        assert u <= U, f"unmasked count {u} exceeds padded capacity {U}"
        idx_pad = np.concatenate([idx, np.full(U - u, idx[0], dtype=np.int64)])
        xqt_ = _tile_pcs(x[b][idx_pad].astype(bf16).T)  # [128, HC, U]
        qchunks = {
            f"xq{j}": np.ascontiguousarray(xqt_[:, :, 512 * j : min(512 * (j + 1), U)])
            for j in range(1, 5)
        }
        # chunk 0 carries [Wq.T | Wv.T] as 128 extra columns
        qchunks["xq0"] = np.ascontiguousarray(
            np.concatenate([xqt_[:, :, 0:512], wqv], axis=2)
        )
        mbv = np.zeros(U, dtype=np.float32)
        mbv[u:] = NEG
        mbt = np.ascontiguousarray(mbv.reshape(UT, 128).T)  # [128, UT]
        for half in range(2):
            xkt_ = _tile_pcs(
                x[b, half * SC : (half + 1) * SC].astype(bf16).T
            )  # [128, HC, SC]
            m = {
                f"xk{k}": np.ascontiguousarray(xkt_[:, :, 512 * k : 512 * (k + 1)])
                for k in range(1, 4)
            }
            # chunk 0 carries [Wk.T | Wk.T] as 128 extra columns
            m["xk0"] = np.ascontiguousarray(
                np.concatenate([xkt_[:, :, 0:512], wk2], axis=2)
            )
            m.update(qchunks)
            m.update({"mb": mbt, "identb": identb, "identf": identf})
            maps.append(m)
    return maps


def kernel(x, mask, Wk, Wq, Wv):
    from concourse.bass_utils import run_bass_kernel_spmd

    if "nc" not in _CACHE:
        _CACHE["nc"] = _build()
    nc = _CACHE["nc"]
    maps = _in_maps(x, mask, Wk, Wq, Wv)
    br = run_bass_kernel_spmd(nc, maps, list(range(N_CORES)))
    out = np.empty((B, S, D), dtype=np.float32)
    for c in range(N_CORES):
        b, half = c // 2, c % 2
        # device layout [128, 16, D]: row 128*k + p lives at [p, k, :]
        o = br.results[c]["out"].transpose(1, 0, 2).reshape(SC, D)
        out[b, half * SC : (half + 1) * SC, :] = o
    return out
